# revision 8
# baseline (speedup 1.0000x reference)
"""Trainium2 Bass kernel for GQA attention (B=4, S=2048, H=576, 9 heads / 3 KV groups, RoPE).

Sharding: 8 cores = (batch b, seq-half) pairs. Each core computes the full
attention output for 1024 query rows of one batch element (keys/values over
the full 2048 positions of that batch element are recomputed locally; no
collectives needed).

Layout strategy: everything stays "transposed" (features on partitions, seq on
free dim):
  QT = wq @ hsT, KT = wk @ hsT (RoPE applied in T space on DVE)
  V natural [s, hv] via lhsT = hsT chunks, stored per-group as [1 | V]
  ST[k, q] = KT.T-stationary @ QT (two heads row-tiled concurrently)
  exp on ACT (2/3 of key chunks) or DVE via int16-Schraudolph bitcast (1/3)
  av[0:65, q] = [1 | V].T @ attnT  (column of ones gives softmax denominator
  in av row 0 -> reciprocal reads PSUM partition 0 directly)
  final^T = woT.T-stationary @ (av[1:65] * recip-broadcast)
Matmul inputs fp16 (fp32 PSUM accumulation), output fp32.

Program order is arranged so attention (ACT/DVE exp work) starts as early as
possible: K proj (both seq pieces) -> Q chunk 0 -> all V chunks -> attention
blocks, with Q chunks 1-4 emitted between attention blocks.
"""

import sys

if "/opt/trn_rl_repo" not in sys.path:
    sys.path.insert(0, "/opt/trn_rl_repo")

import numpy as np

import concourse.bass as bass
import concourse.mybir as mybir
import concourse.tile as tile
from concourse import bacc
from concourse.bass_utils import run_bass_kernel_spmd

F16 = mybir.dt.float16
F32 = mybir.dt.float32
I16 = mybir.dt.int16

B = 4
S = 2048
SQ = 1024  # query rows per core
H = 576
HP = 640  # hidden padded to 5*128
NH = 9
HD = 64
KV = 192
G = 3
ROPE_THETA = 10000.0
SCALE = 1.0 / 8.0  # 1/sqrt(HD)

NDC = HP // 128  # 5 contraction chunks
NEC = 5  # output feature chunks of QT (4*128 + 64)
NKC = S // 128  # 16 key chunks
# head pairs for processing: (0,1),(2,3),(4,5),(6,7),(8,)
PAIRS = [(0, 1), (2, 3), (4, 5), (6, 7), (8,)]

# key chunks whose exp runs on DVE (Schraudolph) instead of ACT
DVE_KCS = frozenset((5, 11))

# Schraudolph fp16 exp: i16 = round(score * K1 + K2); bitcast i16 -> fp16
# approximates exp(score/8) with ~±3% relative error (bias -45 centers it).
SCHR_K1 = 1024.0 * np.log2(np.e) / 8.0
SCHR_K2 = 1024.0 * 15 - 45.0


def _rope_tables():
    inv_freq = 1.0 / (ROPE_THETA ** (np.arange(0, HD, 2, dtype=np.float32) / HD))
    t = np.arange(S, dtype=np.float32)
    freqs = np.einsum("i,j->ij", inv_freq, t)  # [32, S]
    cos32 = np.cos(freqs)
    sin32 = np.sin(freqs)
    cos4 = np.tile(cos32, (4, 1))  # [128, S]
    # sin indexed by the *source* rows of the cross-mul (walrus requires both
    # SBUF inputs of a DVE op to share base partition): the lo output reads
    # hi rows (32-63) and needs -sin there; the hi output reads lo rows (0-31)
    # and needs +sin there.
    sinq = np.concatenate([sin32, -sin32, sin32, -sin32], axis=0)  # [128, S]
    return cos4.astype(np.float16), sinq.astype(np.float16)


def _build_bass():
    nc = bacc.Bacc("TRN2", target_bir_lowering=False)

    hsT = nc.declare_dram_parameter("hsT", [HP, S], F16, isOutput=False)
    wqT = nc.declare_dram_parameter("wqT", [HP, H], F16, isOutput=False)
    wkT = nc.declare_dram_parameter("wkT", [HP, KV], F16, isOutput=False)
    wvT = nc.declare_dram_parameter("wvT", [HP, KV], F16, isOutput=False)
    woT = nc.declare_dram_parameter("woT", [H, H], F16, isOutput=False)
    cos4 = nc.declare_dram_parameter("cos4", [128, S], F16, isOutput=False)
    sinq = nc.declare_dram_parameter("sinq", [128, S], F16, isOutput=False)
    out = nc.declare_dram_parameter("o", [H, SQ], F32, isOutput=True)

    with tile.TileContext(nc) as tc:
        kernel_body(nc, tc, hsT, wqT, wkT, wvT, woT, cos4, sinq, out)

    nc.compile()
    return nc


def kernel_body(nc, tc, hsT, wqT, wkT, wvT, woT, cos4, sinq, out):
    import contextlib

    ctx = contextlib.ExitStack()
    with ctx:
        # ---------------- persistent SBUF pools ----------------
        wpool = ctx.enter_context(tc.tile_pool(name="w", bufs=1))
        qtp = ctx.enter_context(tc.tile_pool(name="qt", bufs=1))
        ktp = ctx.enter_context(tc.tile_pool(name="kt", bufs=1))
        vap = ctx.enter_context(tc.tile_pool(name="va", bufs=1))
        otp = ctx.enter_context(tc.tile_pool(name="ot", bufs=1))
        ropep = ctx.enter_context(tc.tile_pool(name="rope", bufs=2))
        attnp = ctx.enter_context(tc.tile_pool(name="attn", bufs=4))
        miscp = ctx.enter_context(tc.tile_pool(name="misc", bufs=3))

        # ---------------- load inputs to SBUF ----------------
        # DMA issue order matters: K proj starts as soon as wk + hs arrive.
        wk_sb = []
        for dc in range(NDC):
            t = wpool.tile([128, KV], F16, tag=f"wk{dc}", name=f"wk{dc}")
            nc.sync.dma_start(out=t, in_=wkT[dc * 128 : (dc + 1) * 128, :])
            wk_sb.append(t)
        hs_sb = []
        for dc in range(NDC):
            t = wpool.tile([128, S], F16, tag=f"hs{dc}", name=f"hs{dc}")
            nc.sync.dma_start(out=t, in_=hsT[dc * 128 : (dc + 1) * 128, :])
            hs_sb.append(t)
        cos_sb = wpool.tile([128, S], F16, tag="cos")
        nc.sync.dma_start(out=cos_sb, in_=cos4[:, :])
        sin_sb = wpool.tile([128, S], F16, tag="sin")
        nc.sync.dma_start(out=sin_sb, in_=sinq[:, :])
        wq_sb = []
        for dc in range(NDC):
            t = wpool.tile([128, H], F16, tag=f"wq{dc}", name=f"wq{dc}")
            nc.sync.dma_start(out=t, in_=wqT[dc * 128 : (dc + 1) * 128, :])
            wq_sb.append(t)
        wv_sb = []
        for dc in range(NDC):
            t = wpool.tile([128, KV], F16, tag=f"wv{dc}", name=f"wv{dc}")
            nc.sync.dma_start(out=t, in_=wvT[dc * 128 : (dc + 1) * 128, :])
            wv_sb.append(t)
        wo_sb = []
        for ec in range(NEC):
            m = min(128, H - ec * 128)
            t = wpool.tile([128, H], F16, tag=f"wo{ec}", name=f"wo{ec}")
            nc.sync.dma_start(out=t[:m, :], in_=woT[ec * 128 : ec * 128 + m, :])
            wo_sb.append(t)

        # persistent activation tensors
        qt_sb = [qtp.tile([128, SQ], F16, tag=f"qt{c}", name=f"qt{c}") for c in range(NEC)]
        ktd_sb = [ktp.tile([128, S], F16, tag=f"ktd{g}", name=f"ktd{g}") for g in range(G)]
        va_sb = [vap.tile([128, G * 65], F16, tag=f"va{kc}", name=f"va{kc}") for kc in range(NKC)]
        ot_sb = [otp.tile([128, SQ], F16, tag=f"ot{c}", name=f"ot{c}") for c in range(NEC)]

        def rope(dst_writes, src, n_heads, cos_ap, sin_ap, width):
            """Apply RoPE to src [n_heads*64, width] fp16 sbuf tile.

            dst_writes: list of (dst_ap, src_row) per 64-row head giving where
            the rotated head goes. cos_ap/sin_ap are [128, width] slices.
            """
            tc_t = ropep.tile([128, width], F16, tag="ropec")
            tt = ropep.tile([128, width], F16, tag="ropet")
            n = n_heads * 64
            nc.vector.tensor_mul(tc_t[:n], src[:n], cos_ap[:n])
            for h2 in range(n_heads):
                b0 = h2 * 64
                nc.vector.tensor_mul(
                    tt[b0 : b0 + 32], src[b0 + 32 : b0 + 64], sin_ap[b0 + 32 : b0 + 64]
                )
                nc.vector.tensor_mul(
                    tt[b0 + 32 : b0 + 64], src[b0 : b0 + 32], sin_ap[b0 : b0 + 32]
                )
            for dst, row in dst_writes:
                nc.vector.tensor_add(dst, tc_t[row : row + 64], tt[row : row + 64])

        # ---------------- projections ----------------
        # Queries are always hsT columns [0, SQ): cores covering the second
        # seq half pass hsT (and cos/sin) rolled by -SQ columns, which leaves
        # attention invariant (sum over all keys) while keeping one module.
        QO = 0
        with tc.tile_pool(name="ps", bufs=2, space="PSUM") as pj:
            def k_proj(piece, early):
                so = piece * SQ
                for kc_ch, (roff, nh) in enumerate([(0, 2), (128, 1)]):
                    m = nh * 64
                    kps = pj.tile([128, SQ], F32, tag="big", name=f"kps{piece}{kc_ch}")
                    for dc in range(NDC):
                        for sb2 in range(2):
                            nc.tensor.matmul(
                                kps[:m, sb2 * 512 : (sb2 + 1) * 512],
                                lhsT=wk_sb[dc][:, roff : roff + m],
                                rhs=hs_sb[dc][:, so + sb2 * 512 : so + (sb2 + 1) * 512],
                                start=(dc == 0),
                                stop=(dc == NDC - 1),
                            )
                    kraw = ropep.tile([128, SQ], F16, tag="qraw", name="kraw")
                    if early:
                        nc.scalar.copy(kraw[:m], kps[:m])
                    else:
                        nc.vector.tensor_copy(kraw[:m], kps[:m])
                    writes = []
                    for h2 in range(nh):
                        g = kc_ch * 2 + h2
                        writes.append((ktd_sb[g][0:64, so : so + SQ], h2 * 64))
                    rope(writes, kraw, nh, cos_sb[:, so : so + SQ], sin_sb[:, so : so + SQ], SQ)
                # duplicate rows 0-63 -> 64-127 for row-packed score matmuls
                for g in range(G):
                    nc.sync.dma_start(
                        out=ktd_sb[g][64:128, so : so + SQ],
                        in_=ktd_sb[g][0:64, so : so + SQ],
                    )

            def q_proj(c, early):
                m = min(128, H - c * 128)
                nh = m // 64
                qps = pj.tile([128, SQ], F32, tag="big", name=f"qps{c}")
                for dc in range(NDC):
                    for sb2 in range(2):
                        nc.tensor.matmul(
                            qps[:m, sb2 * 512 : (sb2 + 1) * 512],
                            lhsT=wq_sb[dc][:, c * 128 : c * 128 + m],
                            rhs=hs_sb[dc][:, QO + sb2 * 512 : QO + (sb2 + 1) * 512],
                            start=(dc == 0),
                            stop=(dc == NDC - 1),
                        )
                qraw = ropep.tile([128, SQ], F16, tag="qraw")
                if early:
                    nc.scalar.copy(qraw[:m], qps[:m])
                else:
                    nc.vector.tensor_copy(qraw[:m], qps[:m])
                writes = [
                    (qt_sb[c][h2 * 64 : h2 * 64 + 64, :], h2 * 64) for h2 in range(nh)
                ]
                rope(writes, qraw, nh, cos_sb[:, QO : QO + SQ], sin_sb[:, QO : QO + SQ], SQ)

            def v_proj(kc):
                vps = pj.tile([128, SQ], F32, tag="big", name=f"vps{kc}")
                for dc in range(NDC):
                    nc.tensor.matmul(
                        vps[:, :KV],
                        lhsT=hs_sb[dc][:, kc * 128 : (kc + 1) * 128],
                        rhs=wv_sb[dc][:, :],
                        start=(dc == 0),
                        stop=(dc == NDC - 1),
                    )
                vag = va_sb[kc].rearrange("p (g w) -> p g w", g=G)
                # ones in column 64 of each group -> av row 64 = softmax denom
                nc.vector.memset(vag[:, :, 64:65], 1.0)
                # V copies on ACT: it is idle until the first exp
                nc.scalar.copy(vag[:, :, 0:64], vps[:, :KV].rearrange("p (g w) -> p g w", g=G))

            k_proj(0, early=True)
            k_proj(1, early=True)
            q_proj(0, early=True)
            for kc in range(NKC):
                v_proj(kc)

            # ---------------- attention ----------------
            # The qb=0 and qb=1 blocks of each head pair run as two
            # interleaved chains (X/Y): while one chain's av matmuls wait for
            # its exp, the PE runs the other chain's scores/avs -- so ACT/DVE
            # stream exps back-to-back and the PE stays dense (HAM-warm).
            def attn_pair(pi, pair):
                hA = pair[0]
                gA = hA // 3
                two = len(pair) == 2
                hB = pair[1] if two else None
                gB = (hB // 3) if two else None
                c = hA // 2  # qt chunk index
                width = 1024 if two else 512
                av = {}
                for qb in (0, 1):
                    av[(qb, "A")] = pj.tile(
                        [65, 512], F32, tag=f"avA{qb}", bufs=1, name=f"avA{qb}"
                    )
                    if two:
                        av[(qb, "B")] = pj.tile(
                            [65, 512], F32, tag=f"avB{qb}", bufs=1, name=f"avB{qb}"
                        )

                def score(kc, qb):
                    st = pj.tile([128, 1024], F32, tag="big", name="st")
                    nc.tensor.matmul(
                        st[:, 0:512],
                        lhsT=ktd_sb[gA][0:64, kc * 128 : (kc + 1) * 128],
                        rhs=qt_sb[c][0:64, qb * 512 : (qb + 1) * 512],
                        start=True,
                        stop=True,
                    )
                    if two:
                        nc.tensor.matmul(
                            st[:, 512:1024],
                            lhsT=ktd_sb[gB][64:128, kc * 128 : (kc + 1) * 128],
                            rhs=qt_sb[c][64:128, qb * 512 : (qb + 1) * 512],
                            start=True,
                            stop=True,
                        )
                    return st

                def exp_chunk(st, kc):
                    at_t = attnp.tile([128, 1024], F16, tag="at")
                    if kc in DVE_KCS:
                        # Schraudolph exp on DVE (offload from ACT)
                        nc.vector.tensor_scalar(
                            at_t[:, :width].bitcast(I16),
                            st[:, :width],
                            SCHR_K1,
                            SCHR_K2,
                            mybir.AluOpType.mult,
                            mybir.AluOpType.add,
                        )
                    else:
                        nc.scalar.activation(
                            at_t[:, :width],
                            st[:, :width],
                            mybir.ActivationFunctionType.Exp,
                            scale=SCALE,
                        )
                    return at_t

                def av_mms(at_t, kc, qb):
                    nc.tensor.matmul(
                        av[(qb, "A")],
                        lhsT=va_sb[kc][:, gA * 65 : gA * 65 + 65],
                        rhs=at_t[:, 0:512],
                        start=(kc == 0),
                        stop=(kc == NKC - 1),
                    )
                    if two:
                        nc.tensor.matmul(
                            av[(qb, "B")],
                            lhsT=va_sb[kc][:, gB * 65 : gB * 65 + 65],
                            rhs=at_t[:, 512:1024],
                            start=(kc == 0),
                            stop=(kc == NKC - 1),
                        )

                st_x = score(0, 0)
                st_y = score(0, 1)
                for kc in range(NKC):
                    at_x = exp_chunk(st_x, kc)
                    if kc + 1 < NKC:
                        st_x = score(kc + 1, 0)
                    av_mms(at_x, kc, 0)
                    at_y = exp_chunk(st_y, kc)
                    if kc + 1 < NKC:
                        st_y = score(kc + 1, 1)
                    av_mms(at_y, kc, 1)

                # normalize: out^T = av[0:64] / av[64]
                for qb in (0, 1):
                    for h, key in [(hA, "A")] + ([(hB, "B")] if two else []):
                        avt = av[(qb, key)]
                        # custom-DVE ops drop PSUM partition offsets: stage the
                        # denominator row through SBUF first (regular DVE copy).
                        dn = miscp.tile([1, 512], F32, tag="dn")
                        nc.vector.tensor_copy(dn, avt[64:65, :])
                        rd = miscp.tile([1, 512], F32, tag="rd")
                        nc.vector.reciprocal_approx_fast(out=rd, in_=dn)
                        bc = miscp.tile([64, 512], F32, tag="bc")
                        nc.gpsimd.partition_broadcast(bc, rd)
                        row = (h % 2) * 64
                        nc.vector.tensor_mul(
                            ot_sb[h // 2][row : row + 64, qb * 512 : (qb + 1) * 512],
                            avt[0:64, :],
                            bc,
                        )

            for pi, pair in enumerate(PAIRS):
                attn_pair(pi, pair)
                if pi + 1 < NEC:
                    q_proj(pi + 1, early=False)

            # ---------------- output projection ----------------
            for ec in range(NEC):
                m = min(128, H - ec * 128)
                ft = pj.tile([128, SQ], F32, tag="big", name=f"ft{ec}")
                for sb2 in range(2):
                    for cc in range(NEC):
                        k = min(128, H - cc * 128)
                        nc.tensor.matmul(
                            ft[:m, sb2 * 512 : (sb2 + 1) * 512],
                            lhsT=wo_sb[cc][:k, ec * 128 : ec * 128 + m],
                            rhs=ot_sb[cc][:k, sb2 * 512 : (sb2 + 1) * 512],
                            start=(cc == 0),
                            stop=(cc == NEC - 1),
                        )
                fts = miscp.tile([128, SQ], F32, tag="fts", name="fts")
                nc.vector.tensor_copy(fts[:m, :], ft[:m, :])
                nc.sync.dma_start(
                    out=out[ec * 128 : ec * 128 + m, :],
                    in_=fts[:m, :],
                )


_NC_CACHE = {}


def _get_nc():
    if "nc" not in _NC_CACHE:
        _NC_CACHE["nc"] = _build_bass()
    return _NC_CACHE["nc"]


def kernel(hidden_states, wq, wk, wv, wo):
    cos4, sinq = _rope_tables()

    wq16 = np.zeros((HP, H), np.float16)
    wq16[:H] = wq.T.astype(np.float16)
    wk16 = np.zeros((HP, KV), np.float16)
    wk16[:H] = wk.T.astype(np.float16)
    wv16 = np.zeros((HP, KV), np.float16)
    wv16[:H] = wv.T.astype(np.float16)
    wo16 = wo.T.astype(np.float16)

    cos4r = np.roll(cos4, -SQ, axis=1)
    sinqr = np.roll(sinq, -SQ, axis=1)

    in_maps = []
    core_ids = list(range(8))
    for c in core_ids:
        b, half = c // 2, c % 2
        hsT16 = np.zeros((HP, S), np.float16)
        hsT16[:H] = hidden_states[b].T.astype(np.float16)
        if half == 1:
            # roll so this core's queries sit at columns [0, SQ); keys keep
            # their correct rope position via the equally-rolled cos/sin.
            hsT16 = np.roll(hsT16, -SQ, axis=1)
        in_maps.append(
            {
                "hsT": hsT16,
                "wqT": wq16,
                "wkT": wk16,
                "wvT": wv16,
                "woT": wo16,
                "cos4": cos4 if half == 0 else cos4r,
                "sinq": sinq if half == 0 else sinqr,
            }
        )

    global _LAST_IN_MAPS
    _LAST_IN_MAPS = in_maps
    nc = _get_nc()
    res = run_bass_kernel_spmd(nc, in_maps, core_ids=core_ids)

    out = np.empty((B, S, H), np.float32)
    for c in core_ids:
        b, half = c // 2, c % 2
        out[b, half * SQ : (half + 1) * SQ, :] = res.results[c]["o"].T
    return out


if __name__ == "__main__":
    rng = np.random.default_rng(0)
    hs = rng.standard_normal((B, S, H), dtype=np.float32)
    s = 1.0 / np.sqrt(H)
    wq = rng.standard_normal((H, H), dtype=np.float32) * s
    wk = rng.standard_normal((KV, H), dtype=np.float32) * s
    wv = rng.standard_normal((KV, H), dtype=np.float32) * s
    wo = rng.standard_normal((H, H), dtype=np.float32) * s
    o = kernel(hidden_states=hs, wq=wq, wk=wk, wv=wv, wo=wo)
    print(o.shape, o.dtype, np.abs(o).mean())


# revision 10
# speedup vs baseline: 1.3800x; 1.3800x over previous
"""Trainium2 Bass kernel for GQA attention (B=4, S=2048, H=576, 9 heads / 3 KV groups, RoPE).

Sharding: 8 cores = (batch b, seq-half) pairs. Each core computes the full
attention output for 1024 query rows of one batch element (keys/values over
the full 2048 positions of that batch element are recomputed locally; no
collectives needed).

Layout strategy: everything stays "transposed" (features on partitions, seq on
free dim):
  QT = wq @ hsT, KT = wk @ hsT (RoPE applied in T space on DVE)
  V natural [s, hv] via lhsT = hsT chunks, stored per-group as [1 | V]
  ST[k, q] = KT.T-stationary @ QT (two heads row-tiled concurrently)
  exp on ACT (2/3 of key chunks) or DVE via int16-Schraudolph bitcast (1/3)
  av[0:65, q] = [1 | V].T @ attnT  (column of ones gives softmax denominator
  in av row 0 -> reciprocal reads PSUM partition 0 directly)
  final^T = woT.T-stationary @ (av[1:65] * recip-broadcast)
Matmul inputs fp16 (fp32 PSUM accumulation), output fp32.

Program order is arranged so attention (ACT/DVE exp work) starts as early as
possible: K proj (both seq pieces) -> Q chunk 0 -> all V chunks -> attention
blocks, with Q chunks 1-4 emitted between attention blocks.
"""

import sys

if "/opt/trn_rl_repo" not in sys.path:
    sys.path.insert(0, "/opt/trn_rl_repo")

import numpy as np

import concourse.bass as bass
import concourse.mybir as mybir
import concourse.tile as tile
from concourse import bacc
from concourse.bass_utils import run_bass_kernel_spmd

F16 = mybir.dt.float16
F32 = mybir.dt.float32
I16 = mybir.dt.int16

B = 4
S = 2048
SQ = 1024  # query rows per core
H = 576
HP = 640  # hidden padded to 5*128
NH = 9
HD = 64
KV = 192
G = 3
ROPE_THETA = 10000.0
SCALE = 1.0 / 8.0  # 1/sqrt(HD)

NDC = HP // 128  # 5 contraction chunks
NEC = 5  # output feature chunks of QT (4*128 + 64)
NKC = S // 128  # 16 key chunks
# head pairs for processing: (0,1),(2,3),(4,5),(6,7),(8,)
PAIRS = [(0, 1), (2, 3), (4, 5), (6, 7), (8,)]

# key chunks whose exp runs on DVE (Schraudolph) instead of ACT
DVE_KCS = frozenset((5, 11))

# Schraudolph fp16 exp: i16 = round(score * K1 + K2); bitcast i16 -> fp16
# approximates exp(score/8) with ~±3% relative error (bias -45 centers it).
SCHR_K1 = 1024.0 * np.log2(np.e) / 8.0
SCHR_K2 = 1024.0 * 15 - 45.0


def _rope_tables():
    inv_freq = 1.0 / (ROPE_THETA ** (np.arange(0, HD, 2, dtype=np.float32) / HD))
    t = np.arange(S, dtype=np.float32)
    freqs = np.einsum("i,j->ij", inv_freq, t)  # [32, S]
    cos32 = np.cos(freqs)
    sin32 = np.sin(freqs)
    cos4 = np.tile(cos32, (4, 1))  # [128, S]
    # sin indexed by the *source* rows of the cross-mul (walrus requires both
    # SBUF inputs of a DVE op to share base partition): the lo output reads
    # hi rows (32-63) and needs -sin there; the hi output reads lo rows (0-31)
    # and needs +sin there.
    sinq = np.concatenate([sin32, -sin32, sin32, -sin32], axis=0)  # [128, S]
    return cos4.astype(np.float16), sinq.astype(np.float16)


def _build_bass():
    nc = bacc.Bacc("TRN2", target_bir_lowering=False)

    hsT = nc.declare_dram_parameter("hsT", [HP, S], F16, isOutput=False)
    wqT = nc.declare_dram_parameter("wqT", [HP, H], F16, isOutput=False)
    wkT = nc.declare_dram_parameter("wkT", [HP, KV], F16, isOutput=False)
    wvT = nc.declare_dram_parameter("wvT", [HP, KV], F16, isOutput=False)
    woT = nc.declare_dram_parameter("woT", [H, H], F16, isOutput=False)
    cos4 = nc.declare_dram_parameter("cos4", [128, S], F16, isOutput=False)
    sinq = nc.declare_dram_parameter("sinq", [128, S], F16, isOutput=False)
    out = nc.declare_dram_parameter("o", [H, SQ], F32, isOutput=True)

    with tile.TileContext(nc) as tc:
        kernel_body(nc, tc, hsT, wqT, wkT, wvT, woT, cos4, sinq, out)

    nc.compile()
    return nc


def kernel_body(nc, tc, hsT, wqT, wkT, wvT, woT, cos4, sinq, out):
    import contextlib

    ctx = contextlib.ExitStack()
    with ctx:
        # ---------------- persistent SBUF pools ----------------
        wpool = ctx.enter_context(tc.tile_pool(name="w", bufs=1))
        qtp = ctx.enter_context(tc.tile_pool(name="qt", bufs=1))
        ktp = ctx.enter_context(tc.tile_pool(name="kt", bufs=1))
        vap = ctx.enter_context(tc.tile_pool(name="va", bufs=1))
        otp = ctx.enter_context(tc.tile_pool(name="ot", bufs=1))
        ropep = ctx.enter_context(tc.tile_pool(name="rope", bufs=2))
        attnp = ctx.enter_context(tc.tile_pool(name="attn", bufs=4))
        miscp = ctx.enter_context(tc.tile_pool(name="misc", bufs=3))

        # ---------------- load inputs to SBUF ----------------
        # DMA issue order matters: K proj starts as soon as wk + hs arrive.
        wk_sb = []
        for dc in range(NDC):
            t = wpool.tile([128, KV], F16, tag=f"wk{dc}", name=f"wk{dc}")
            nc.sync.dma_start(out=t, in_=wkT[dc * 128 : (dc + 1) * 128, :])
            wk_sb.append(t)
        hs_sb = []
        for dc in range(NDC):
            t = wpool.tile([128, S], F16, tag=f"hs{dc}", name=f"hs{dc}")
            nc.sync.dma_start(out=t, in_=hsT[dc * 128 : (dc + 1) * 128, :])
            hs_sb.append(t)
        cos_sb = wpool.tile([128, S], F16, tag="cos")
        nc.sync.dma_start(out=cos_sb, in_=cos4[:, :])
        sin_sb = wpool.tile([128, S], F16, tag="sin")
        nc.sync.dma_start(out=sin_sb, in_=sinq[:, :])
        wq_sb = []
        for dc in range(NDC):
            t = wpool.tile([128, H], F16, tag=f"wq{dc}", name=f"wq{dc}")
            nc.sync.dma_start(out=t, in_=wqT[dc * 128 : (dc + 1) * 128, :])
            wq_sb.append(t)
        wv_sb = []
        for dc in range(NDC):
            t = wpool.tile([128, KV], F16, tag=f"wv{dc}", name=f"wv{dc}")
            nc.sync.dma_start(out=t, in_=wvT[dc * 128 : (dc + 1) * 128, :])
            wv_sb.append(t)
        wo_sb = []
        for ec in range(NEC):
            m = min(128, H - ec * 128)
            t = wpool.tile([128, H], F16, tag=f"wo{ec}", name=f"wo{ec}")
            nc.sync.dma_start(out=t[:m, :], in_=woT[ec * 128 : ec * 128 + m, :])
            wo_sb.append(t)

        # persistent activation tensors
        qt_sb = [qtp.tile([128, SQ], F16, tag=f"qt{c}", name=f"qt{c}") for c in range(NEC)]
        ktd_sb = [ktp.tile([128, S], F16, tag=f"ktd{g}", name=f"ktd{g}") for g in range(G)]
        va_sb = [vap.tile([128, G * 65], F16, tag=f"va{kc}", name=f"va{kc}") for kc in range(NKC)]
        ot_sb = [otp.tile([128, SQ], F16, tag=f"ot{c}", name=f"ot{c}") for c in range(NEC)]

        def rope(dst_writes, src, n_heads, cos_ap, sin_ap, width):
            """Apply RoPE to src [n_heads*64, width] fp16 sbuf tile.

            dst_writes: list of (dst_ap, src_row) per 64-row head giving where
            the rotated head goes. cos_ap/sin_ap are [128, width] slices.
            """
            tc_t = ropep.tile([128, width], F16, tag="ropec")
            tt = ropep.tile([128, width], F16, tag="ropet")
            n = n_heads * 64
            nc.vector.tensor_mul(tc_t[:n], src[:n], cos_ap[:n])
            for h2 in range(n_heads):
                b0 = h2 * 64
                nc.vector.tensor_mul(
                    tt[b0 : b0 + 32], src[b0 + 32 : b0 + 64], sin_ap[b0 + 32 : b0 + 64]
                )
                nc.vector.tensor_mul(
                    tt[b0 + 32 : b0 + 64], src[b0 : b0 + 32], sin_ap[b0 : b0 + 32]
                )
            for dst, row in dst_writes:
                nc.vector.tensor_add(dst, tc_t[row : row + 64], tt[row : row + 64])

        # ---------------- projections ----------------
        # Queries are always hsT columns [0, SQ): cores covering the second
        # seq half pass hsT (and cos/sin) rolled by -SQ columns, which leaves
        # attention invariant (sum over all keys) while keeping one module.
        QO = 0
        with tc.tile_pool(name="ps", bufs=2, space="PSUM") as pj:
            def k_proj(piece, early):
                so = piece * SQ
                for kc_ch, (roff, nh) in enumerate([(0, 2), (128, 1)]):
                    m = nh * 64
                    kps = pj.tile([128, SQ], F32, tag="big", name=f"kps{piece}{kc_ch}")
                    for dc in range(NDC):
                        for sb2 in range(2):
                            nc.tensor.matmul(
                                kps[:m, sb2 * 512 : (sb2 + 1) * 512],
                                lhsT=wk_sb[dc][:, roff : roff + m],
                                rhs=hs_sb[dc][:, so + sb2 * 512 : so + (sb2 + 1) * 512],
                                start=(dc == 0),
                                stop=(dc == NDC - 1),
                            )
                    kraw = ropep.tile([128, SQ], F16, tag="qraw", name="kraw")
                    if early:
                        nc.scalar.copy(kraw[:m], kps[:m])
                    else:
                        nc.vector.tensor_copy(kraw[:m], kps[:m])
                    writes = []
                    for h2 in range(nh):
                        g = kc_ch * 2 + h2
                        writes.append((ktd_sb[g][0:64, so : so + SQ], h2 * 64))
                    rope(writes, kraw, nh, cos_sb[:, so : so + SQ], sin_sb[:, so : so + SQ], SQ)
                # duplicate rows 0-63 -> 64-127 for row-packed score matmuls
                for g in range(G):
                    nc.sync.dma_start(
                        out=ktd_sb[g][64:128, so : so + SQ],
                        in_=ktd_sb[g][0:64, so : so + SQ],
                    )

            def q_proj(c, early):
                m = min(128, H - c * 128)
                nh = m // 64
                qps = pj.tile([128, SQ], F32, tag="big", name=f"qps{c}")
                for dc in range(NDC):
                    for sb2 in range(2):
                        nc.tensor.matmul(
                            qps[:m, sb2 * 512 : (sb2 + 1) * 512],
                            lhsT=wq_sb[dc][:, c * 128 : c * 128 + m],
                            rhs=hs_sb[dc][:, QO + sb2 * 512 : QO + (sb2 + 1) * 512],
                            start=(dc == 0),
                            stop=(dc == NDC - 1),
                        )
                qraw = ropep.tile([128, SQ], F16, tag="qraw")
                if early:
                    nc.scalar.copy(qraw[:m], qps[:m])
                else:
                    nc.vector.tensor_copy(qraw[:m], qps[:m])
                writes = [
                    (qt_sb[c][h2 * 64 : h2 * 64 + 64, :], h2 * 64) for h2 in range(nh)
                ]
                rope(writes, qraw, nh, cos_sb[:, QO : QO + SQ], sin_sb[:, QO : QO + SQ], SQ)

            def v_proj(kc):
                vps = pj.tile([128, SQ], F32, tag="big", name=f"vps{kc}")
                for dc in range(NDC):
                    nc.tensor.matmul(
                        vps[:, :KV],
                        lhsT=hs_sb[dc][:, kc * 128 : (kc + 1) * 128],
                        rhs=wv_sb[dc][:, :],
                        start=(dc == 0),
                        stop=(dc == NDC - 1),
                    )
                vag = va_sb[kc].rearrange("p (g w) -> p g w", g=G)
                # ones in column 64 of each group -> av row 64 = softmax denom
                nc.vector.memset(vag[:, :, 64:65], 1.0)
                # V copies on ACT: it is idle until the first exp
                nc.scalar.copy(vag[:, :, 0:64], vps[:, :KV].rearrange("p (g w) -> p g w", g=G))

            k_proj(0, early=True)
            k_proj(1, early=True)
            q_proj(0, early=True)
            for kc in range(NKC):
                v_proj(kc)

            # ---------------- attention ----------------
            # The qb=0 and qb=1 blocks of each head pair run as two
            # interleaved chains (X/Y): while one chain's av matmuls wait for
            # its exp, the PE runs the other chain's scores/avs -- so ACT/DVE
            # stream exps back-to-back and the PE stays dense (HAM-warm).
            def attn_pair(pi, pair):
                hA = pair[0]
                gA = hA // 3
                two = len(pair) == 2
                hB = pair[1] if two else None
                gB = (hB // 3) if two else None
                c = hA // 2  # qt chunk index
                width = 1024 if two else 512
                av = {}
                for qb in (0, 1):
                    av[(qb, "A")] = pj.tile(
                        [65, 512], F32, tag=f"avA{qb}", bufs=1, name=f"avA{qb}"
                    )
                    if two:
                        av[(qb, "B")] = pj.tile(
                            [65, 512], F32, tag=f"avB{qb}", bufs=1, name=f"avB{qb}"
                        )

                def score(kc, qb):
                    st = pj.tile([128, 1024], F32, tag="big", name="st")
                    nc.tensor.matmul(
                        st[:, 0:512],
                        lhsT=ktd_sb[gA][0:64, kc * 128 : (kc + 1) * 128],
                        rhs=qt_sb[c][0:64, qb * 512 : (qb + 1) * 512],
                        start=True,
                        stop=True,
                    )
                    if two:
                        nc.tensor.matmul(
                            st[:, 512:1024],
                            lhsT=ktd_sb[gB][64:128, kc * 128 : (kc + 1) * 128],
                            rhs=qt_sb[c][64:128, qb * 512 : (qb + 1) * 512],
                            start=True,
                            stop=True,
                        )
                    return st

                def exp_chunk(st, kc):
                    at_t = attnp.tile([128, 1024], F16, tag="at")
                    if kc in DVE_KCS:
                        # Schraudolph exp on DVE (offload from ACT)
                        nc.vector.tensor_scalar(
                            at_t[:, :width].bitcast(I16),
                            st[:, :width],
                            SCHR_K1,
                            SCHR_K2,
                            mybir.AluOpType.mult,
                            mybir.AluOpType.add,
                        )
                    else:
                        nc.scalar.activation(
                            at_t[:, :width],
                            st[:, :width],
                            mybir.ActivationFunctionType.Exp,
                            scale=SCALE,
                        )
                    return at_t

                def av_mms(at_t, kc, qb):
                    nc.tensor.matmul(
                        av[(qb, "A")],
                        lhsT=va_sb[kc][:, gA * 65 : gA * 65 + 65],
                        rhs=at_t[:, 0:512],
                        start=(kc == 0),
                        stop=(kc == NKC - 1),
                    )
                    if two:
                        nc.tensor.matmul(
                            av[(qb, "B")],
                            lhsT=va_sb[kc][:, gB * 65 : gB * 65 + 65],
                            rhs=at_t[:, 512:1024],
                            start=(kc == 0),
                            stop=(kc == NKC - 1),
                        )

                st_x = score(0, 0)
                st_y = score(0, 1)
                for kc in range(NKC):
                    at_x = exp_chunk(st_x, kc)
                    if kc + 1 < NKC:
                        st_x = score(kc + 1, 0)
                    av_mms(at_x, kc, 0)
                    at_y = exp_chunk(st_y, kc)
                    if kc + 1 < NKC:
                        st_y = score(kc + 1, 1)
                    av_mms(at_y, kc, 1)

                # normalize: out^T = av[0:64] / av[64]. Emit the per-head
                # chains interleaved so the gpsimd broadcasts overlap the DVE
                # dn/recip work instead of serializing head-by-head.
                heads = [
                    (qb, h, key)
                    for qb in (0, 1)
                    for h, key in [(hA, "A")] + ([(hB, "B")] if two else [])
                ]
                bcs = {}

                def emit_mul(qb, h, key):
                    row = (h % 2) * 64
                    nc.vector.tensor_mul(
                        ot_sb[h // 2][row : row + 64, qb * 512 : (qb + 1) * 512],
                        av[(qb, key)][0:64, :],
                        bcs[(qb, key)],
                    )

                for i, (qb, h, key) in enumerate(heads):
                    avt = av[(qb, key)]
                    # custom-DVE ops drop PSUM partition offsets: stage the
                    # denominator row through SBUF first (regular DVE copy).
                    dn = miscp.tile([1, 512], F32, tag="dn")
                    nc.vector.tensor_copy(dn, avt[64:65, :])
                    rd = miscp.tile([1, 512], F32, tag="rd")
                    nc.vector.reciprocal_approx_fast(out=rd, in_=dn)
                    bc = miscp.tile([64, 512], F32, tag=f"bc{qb}{key}")
                    nc.gpsimd.partition_broadcast(bc, rd)
                    bcs[(qb, key)] = bc
                    if i >= 1:
                        emit_mul(*heads[i - 1])
                emit_mul(*heads[-1])

            # q_proj for pair c+1 is emitted BEFORE pair c so its rope (DVE)
            # runs during pair c's early chunks, not at the pair boundary
            # behind the normalization chains.
            q_proj(1, early=True)
            for pi, pair in enumerate(PAIRS):
                if pi + 2 < NEC:
                    q_proj(pi + 2, early=False)
                attn_pair(pi, pair)

            # ---------------- output projection ----------------
            for ec in range(NEC):
                m = min(128, H - ec * 128)
                ft = pj.tile([128, SQ], F32, tag="big", name=f"ft{ec}")
                for sb2 in range(2):
                    for cc in range(NEC):
                        k = min(128, H - cc * 128)
                        nc.tensor.matmul(
                            ft[:m, sb2 * 512 : (sb2 + 1) * 512],
                            lhsT=wo_sb[cc][:k, ec * 128 : ec * 128 + m],
                            rhs=ot_sb[cc][:k, sb2 * 512 : (sb2 + 1) * 512],
                            start=(cc == 0),
                            stop=(cc == NEC - 1),
                        )
                fts = miscp.tile([128, SQ], F32, tag="fts", name="fts")
                nc.vector.tensor_copy(fts[:m, :], ft[:m, :])
                nc.sync.dma_start(
                    out=out[ec * 128 : ec * 128 + m, :],
                    in_=fts[:m, :],
                )


_NC_CACHE = {}


def _get_nc():
    if "nc" not in _NC_CACHE:
        _NC_CACHE["nc"] = _build_bass()
    return _NC_CACHE["nc"]


def kernel(hidden_states, wq, wk, wv, wo):
    cos4, sinq = _rope_tables()

    wq16 = np.zeros((HP, H), np.float16)
    wq16[:H] = wq.T.astype(np.float16)
    wk16 = np.zeros((HP, KV), np.float16)
    wk16[:H] = wk.T.astype(np.float16)
    wv16 = np.zeros((HP, KV), np.float16)
    wv16[:H] = wv.T.astype(np.float16)
    wo16 = wo.T.astype(np.float16)

    cos4r = np.roll(cos4, -SQ, axis=1)
    sinqr = np.roll(sinq, -SQ, axis=1)

    in_maps = []
    core_ids = list(range(8))
    for c in core_ids:
        b, half = c // 2, c % 2
        hsT16 = np.zeros((HP, S), np.float16)
        hsT16[:H] = hidden_states[b].T.astype(np.float16)
        if half == 1:
            # roll so this core's queries sit at columns [0, SQ); keys keep
            # their correct rope position via the equally-rolled cos/sin.
            hsT16 = np.roll(hsT16, -SQ, axis=1)
        in_maps.append(
            {
                "hsT": hsT16,
                "wqT": wq16,
                "wkT": wk16,
                "wvT": wv16,
                "woT": wo16,
                "cos4": cos4 if half == 0 else cos4r,
                "sinq": sinq if half == 0 else sinqr,
            }
        )

    global _LAST_IN_MAPS
    _LAST_IN_MAPS = in_maps
    nc = _get_nc()
    res = run_bass_kernel_spmd(nc, in_maps, core_ids=core_ids)

    out = np.empty((B, S, H), np.float32)
    for c in core_ids:
        b, half = c // 2, c % 2
        out[b, half * SQ : (half + 1) * SQ, :] = res.results[c]["o"].T
    return out


if __name__ == "__main__":
    rng = np.random.default_rng(0)
    hs = rng.standard_normal((B, S, H), dtype=np.float32)
    s = 1.0 / np.sqrt(H)
    wq = rng.standard_normal((H, H), dtype=np.float32) * s
    wk = rng.standard_normal((KV, H), dtype=np.float32) * s
    wv = rng.standard_normal((KV, H), dtype=np.float32) * s
    wo = rng.standard_normal((H, H), dtype=np.float32) * s
    o = kernel(hidden_states=hs, wq=wq, wk=wk, wv=wv, wo=wo)
    print(o.shape, o.dtype, np.abs(o).mean())
